# revision 27
# baseline (speedup 1.0000x reference)
"""Trainium2 Bass kernel for windowed multi-head attention with dynamic
position bias (sparse_attention, B=2, H=W=256, 8x32 windows, 6 heads, d=32).

v2: three-engine softmax. The baseline was simultaneously ScalarE-bound
(exp of all 6x256x256 logits/window, 201us busy) and PE-bound (bias
identity-matmuls = 44% of PE streaming). This version:
  - replaces the fp16 bias identity-matmuls with fp8 DoubleRow matmuls
    (identity|identity stationary, coarse|residual moving) at 0.5
    cycles/column -- half the PE streaming cost, ~3e-4 bias error.
  - splits the exp work: ScalarE does true exp for heads {0,1} of each
    3-head PSUM group (2048 cols/window); head 2 (1024 cols/window) uses
    a Schraudolph-style bit-trick: Pool adds a magic constant to the PSUM
    logits (pre-scaled by 1024*log2e via Q) and converts to int16 = the
    fp16 bit pattern of 2^i(1+f); one custom DVE uop-pipeline op applies
    a minimax quadratic mantissa correction (rel err <= 5.4e-3 on that
    1/3 slice, exact elsewhere) straight into the fp16 P tile.
  - PV + reciprocal-normalize unchanged in spirit; output fp16.
"""

import sys

sys.path.insert(0, "/opt/trn_rl_repo")

import numpy as np

import concourse.bass as bass
import concourse.tile as tile
from concourse import mybir
from concourse.alu_op_type import AluOpType
from concourse.bass_utils import run_bass_kernel_spmd

F32 = mybir.dt.float32
BF16 = mybir.dt.bfloat16
FP16 = mybir.dt.float16
FP8 = mybir.dt.float8e4
I16 = mybir.dt.int16
EXP = mybir.ActivationFunctionType.Exp

N_CORES = 8
B, H, W = 2, 256, 256
H_SP, W_SP = 8, 32
NUM_HEADS = 6
DIM = 192
HEAD_DIM = 32
SCALE = HEAD_DIM ** -0.5
N = H_SP * W_SP                     # 256 tokens / window
NW_TOTAL = B * (H // H_SP) * (W // W_SP)   # 512 windows
NW = NW_TOTAL // N_CORES            # 64 windows / core

# Schraudolph exp2 path: PSUM holds t = LAM*(s+bias); u = int16(t + MAGIC)
# bitcast fp16 is 2^(i-15)(1+f); P = u * ((m + PC1)*m + PC2) with m = 1+f
# extracted from the fp32 bit pattern ((bits|0x3F800000)&0x3FFFFFFF).
LAM = 1024.0 * np.log2(np.e)        # 1477.3195458...
MAGIC = 13205.898538354311
PC1 = -2.9537455388278904
PC2 = 6.228467047720157
MASK_CONST = float(np.int32(0x3FFFFFFF).view(np.float32))
ACT_SCALE = float(np.log(2.0) / 1024.0)

# per 3-head PSUM group [128, 1536]: cols [0,ACT_COLS) get ScalarE true
# exp; the rest go through the DVE int16 bit-trick (GPSIMD cannot touch
# PSUM on trn2, so both legs of the split are ScalarE/DVE). The fp8
# DoubleRow bias matmuls only cover [0,DR_COLS) -- the S matmuls there
# accumulate (start=False); past it they start fresh and the bias rides
# the DVE tensor_scalar's in1 (zero-padded over [ACT_COLS,DR_COLS)).
ACT_COLS = 1248
STT_COLS = 1536 - ACT_COLS
DR_COLS = 1280                      # banks 0,1 fully + bank 2 kc0 region


# --------------------------------------------------------------------------
# custom DVE op: out = in0 * ((m + s1)*m + imm2), m = 1+frac(mantissa(in0))
# --------------------------------------------------------------------------
def _register_custom_ops():
    from concourse import dve_ops as DO
    from concourse.dve_spec import (Spec, Src0, Src1, C0, C1, C2, One, Bin,
                                    lower, _has_src1)
    from concourse.dve_uop import AluOp, DveOpSpec

    def _reg(name, spec):
        if name in DO._SUB_OPCODE_FOR_NAME:
            return next(op for op in DO.OPS if op.name == name)
        row = DO._CUSTOM_DVE_ROW_BASE + len(DO.OPS)
        sha = DveOpSpec(name=name, opcode=row, uops=lower(spec, ver="v3"),
                        rd1_en=_has_src1(spec)).sha("v3")
        op = DO.DveOp(name, spec, subdim=False, uops_sha={"v3": sha})
        DO.OPS.append(op)
        DO._SUB_OPCODE_FOR_NAME[name] = row
        DO.CUSTOM_DVE_SPECS[name] = spec
        return op

    # exp2 mantissa fixup: out = in0 * ((m + s1)*m + imm2),
    # m = 1+frac extracted from the fp32 bit pattern of in0.
    _m = Bin(AluOp.BITWISE_AND, Bin(AluOp.BITWISE_OR, Src0, One), C0)

    def _fix_ref(in0, in1, s0, s1, imm2):
        x = np.ascontiguousarray(in0.astype(np.float32))
        bits = x.view(np.int32)
        mm = ((bits | 0x3F800000) & 0x3FFFFFFF).view(np.float32)
        return (x * ((mm + s1) * mm + imm2)).astype(np.float32)

    fixup = _reg("EXP2_FIXUP_ANT",
                 Spec(body=Src0 * ((_m + C1) * _m + C2), reference=_fix_ref))

    # fused normalize: out = in0 * recip_approx(in1); BITWISE_NOT seed +
    # one Newton step (~0.4% max rel err, cancels nothing downstream).
    _ny = Bin(AluOp.BITWISE_NOT, Src1, Src1)
    _y0 = _ny * C0
    _y1 = _y0 * (C1 - Src1 * _y0)

    def _nrm_ref(in0, in1, s0, s1, imm2):
        x1 = np.ascontiguousarray(in1.astype(np.float32))
        ny = (~x1.view(np.int32)).view(np.float32)
        y0 = ny * np.float32(s0)
        y1 = y0 * (np.float32(s1) - x1 * y0)
        return (in0.astype(np.float32) * y1).astype(np.float32)

    nrm = _reg("NORM_RECIP_ANT", Spec(body=Src0 * _y1, reference=_nrm_ref))
    return fixup, nrm


EXP2_FIXUP, NORM_RECIP = _register_custom_ops()
# Chebyshev seed constants for x*bitcast(~x) in [-4.5, -4] (see dve_ops)
NR_C0 = -0.23549792
NR_C1 = 2.0017324


# --------------------------------------------------------------------------
# device program
# --------------------------------------------------------------------------
WG = 8     # windows per input slab
OG = 4     # windows per output slab


def build_program(nw=NW):
    from concourse import bacc
    nc = bacc.Bacc("TRN2", target_bir_lowering=False, debug=False)

    qT = nc.dram_tensor("qT", [DIM, nw * N], FP16, kind="ExternalInput").ap()
    kT = nc.dram_tensor("kT", [DIM, nw * N], FP16, kind="ExternalInput").ap()
    vA = nc.dram_tensor("vA", [128, nw * 396], FP16, kind="ExternalInput").ap()
    biasDR = nc.dram_tensor("biasDR", [128, 2 * DR_COLS], FP16,
                            kind="ExternalInput").ap()
    biasL = nc.dram_tensor("biasL", [128, 2 * STT_COLS], FP16,
                           kind="ExternalInput").ap()
    id2 = nc.dram_tensor("id2", [128, 128], FP16, kind="ExternalInput").ap()
    outw = nc.dram_tensor("outw", [128, nw * 2 * DIM], FP16,
                          kind="ExternalOutput").ap()

    with tile.TileContext(nc) as tc:
        _emit(nc, tc, nw, qT, kT, vA, biasDR, biasL, id2, outw)
    nc.compile()
    return nc


def _emit(nc, tc, nw, qT, kT, vA, biasDR, biasL, id2, outw):
    from contextlib import ExitStack
    ctx = ExitStack()

    # residents: fp8 DoubleRow bias (coarse|residual per covered bank
    # range, LAM-scaled), the doubled identity stationary, and the
    # zero-padded fp16 bias for the DVE slice.
    bdr_sb = nc.alloc_sbuf_tensor("bdr_sb", [128, 2 * DR_COLS], FP16).ap()
    bl_sb = nc.alloc_sbuf_tensor("bl_sb", [128, 2 * STT_COLS], FP16).ap()
    id2_sb = nc.alloc_sbuf_tensor("id2_sb", [128, 128], FP16).ap()
    nc.sync.dma_start(bdr_sb, biasDR)
    nc.sync.dma_start(bl_sb, biasL)
    nc.sync.dma_start(id2_sb, id2)

    pin = ctx.enter_context(tc.tile_pool(name="pin", bufs=2))
    pps = ctx.enter_context(tc.tile_pool(name="pps", bufs=2, space="PSUM"))
    ppt = ctx.enter_context(tc.tile_pool(name="ppt", bufs=2))
    pu = ctx.enter_context(tc.tile_pool(name="pu", bufs=2))
    pout = ctx.enter_context(tc.tile_pool(name="pout", bufs=4))

    qa = qb = ka = kb = va = None
    obh = [None]
    pend = []        # (pt, va, wv, w) queue; PV runs two windows behind

    def emit_pv(nc, state):
        pt, pva, pwv, pw = state
        if pw % OG == 0:
            obn = pout.tile([128, OG * 2 * DIM], FP16, tag="ob", bufs=3)
            obh[0] = obn
        ob = obh[0]
        pv = pps.tile([128, 396], F32, tag="pv", bufs=2)
        for qc in (0, 1):
            for h in range(NUM_HEADS):
                base = 1536 * (h // 3) + 512 * (h % 3)
                for kc in (0, 1):
                    nc.tensor.matmul(
                        pv[:, 198 * qc + 33 * h: 198 * qc + 33 * h + 33],
                        lhsT=pt[:, base + 256 * kc + 128 * qc:
                                base + 256 * kc + 128 * qc + 128],
                        rhs=pva[:, pwv + 198 * kc + 33 * h:
                                pwv + 198 * kc + 33 * h + 33],
                        start=(kc == 0), stop=(kc == 1),
                        skip_group_check=True,
                    )
        # normalize: ob = pv * recip(rowsum), rowsum in col 32
        pv3 = pv.rearrange("p (g c) -> p g c", c=33)
        rv = pout.tile([128, 16], F32, tag="rv", bufs=4)
        nc.vector.reciprocal_approx_fast(rv[:, 0:12], pv3[:, :, 32])
        oslot = ob[:, (pw % OG) * 2 * DIM: (pw % OG) * 2 * DIM + 2 * DIM]
        nc.vector.tensor_tensor(
            oslot.rearrange("p (g c) -> p g c", c=32),
            pv3[:, :, 0:32],
            rv[:, 0:12].unsqueeze(-1).broadcast_to([128, 12, 32]),
            op=AluOpType.mult,
        )
        if pw == nw - 3 and nw >= 8 and OG == 4:   # early half-flush
            base = (pw - 1) * 2 * DIM
            nc.sync.dma_start(outw[:, base: base + 2 * 2 * DIM],
                              ob[:, 0: 2 * 2 * DIM])
        elif pw == nw - 1 and nw >= 8 and OG == 4:
            base = (pw - 1) * 2 * DIM
            nc.sync.dma_start(outw[:, base: base + 2 * 2 * DIM],
                              ob[:, 2 * 2 * DIM: 4 * 2 * DIM])
        elif pw % OG == OG - 1:
            base = (pw - (OG - 1)) * 2 * DIM
            nc.sync.dma_start(outw[:, base: base + OG * 2 * DIM], ob)

    slabs = [(0, 1), (1, 1), (2, 2), (4, 4)] + \
        [(s, WG) for s in range(WG, nw, WG)]
    slab_of = {}
    for s0, sn in slabs:
        for i in range(sn):
            slab_of[s0 + i] = s0

    for w in range(nw):
        if slab_of[w] == w:
            sn = dict(slabs)[w]
            g = w * N
            qa = pin.tile([128, WG * N], FP16, tag="qa",
                          padded_shape=[128, WG * N])
            nc.sync.dma_start(qa[:, 0:sn * N], qT[0:128, g:g + sn * N])
            qb = pin.tile([64, WG * N], FP16, tag="qb",
                          padded_shape=[64, WG * N])
            nc.sync.dma_start(qb[:, 0:sn * N], qT[128:192, g:g + sn * N])
            ka = pin.tile([128, WG * N], FP16, tag="ka",
                          padded_shape=[128, WG * N])
            nc.sync.dma_start(ka[:, 0:sn * N], kT[0:128, g:g + sn * N])
            kb = pin.tile([64, WG * N], FP16, tag="kb",
                          padded_shape=[64, WG * N])
            nc.sync.dma_start(kb[:, 0:sn * N], kT[128:192, g:g + sn * N])
            va = pin.tile([128, WG * 396], FP16, tag="va",
                          padded_shape=[128, WG * 396])
            nc.sync.dma_start(va[:, 0:sn * 396],
                              vA[:, w * 396:(w + sn) * 396])
        wq = (w - slab_of[w]) * N
        wv = (w - slab_of[w]) * 396

        pt = ppt.tile([128, 3072], FP16, tag="pt", bufs=3)
        u16 = pu.tile([128, 2 * STT_COLS], I16, tag="u16")
        uf16 = u16.bitcast(FP16)

        for grp in range(2):
            s = pps.tile([128, 1536], F32, tag="s")
            heads = (0, 1, 2) if grp == 0 else (3, 4, 5)

            # bias lands first via fp8 DoubleRow identity matmuls (coarse
            # + residual halves) over [0, DR_COLS), then K'Q accumulates;
            # past DR_COLS the S matmul starts fresh.
            dr_off = DR_COLS * grp
            for hh, cov in ((0, 512), (1, 512), (2, DR_COLS - 1024)):
                nc.tensor.matmul(
                    s[:, 512 * hh: 512 * hh + cov],
                    lhsT=id2_sb,
                    rhs=bdr_sb[:, dr_off + 512 * hh:
                               dr_off + 512 * hh + cov],
                    start=True, stop=False, skip_group_check=True,
                )
            for kc in (0, 1):
                for h in heads:
                    hh = h % 3
                    hp = h if h < 4 else h - 4
                    ktile = ka if h < 4 else kb
                    qtile = qa if h < 4 else qb
                    nc.tensor.matmul(
                        s[:, 512 * hh + 256 * kc: 512 * hh + 256 * kc + 256],
                        lhsT=ktile[32 * hp: 32 * hp + 32,
                                   wq + 128 * kc: wq + 128 * kc + 128],
                        rhs=qtile[32 * hp: 32 * hp + 32, wq: wq + N],
                        start=(512 * hh + 256 * kc >= DR_COLS),
                        stop=(kc == 1),
                        tile_position=(32 * hp, 0), skip_group_check=True,
                    )

            # exp: ScalarE true exp on the DR-biased prefix; DVE bit-trick
            # on the rest (add magic + in1 bias tail, convert to int16,
            # then the custom DVE op applies the mantissa fixup into pt).
            nc.scalar.activation(pt[:, 1536 * grp: 1536 * grp + ACT_COLS],
                                 s[:, 0:ACT_COLS], EXP, scale=ACT_SCALE)
            nc.vector.scalar_tensor_tensor(
                u16[:, STT_COLS * grp: STT_COLS * grp + STT_COLS],
                s[:, ACT_COLS:1536], MAGIC,
                bl_sb[:, STT_COLS * grp: STT_COLS * grp + STT_COLS],
                op0=AluOpType.add, op1=AluOpType.add,
            )
            nc.vector._custom_dve(
                EXP2_FIXUP,
                out=pt[:, 1536 * grp + ACT_COLS: 1536 * grp + 1536],
                in0=uf16[:, STT_COLS * grp: STT_COLS * grp + STT_COLS],
                s0=MASK_CONST, s1=PC1, imm2=PC2,
            )

        pend.append((pt, va, wv, w))
        if len(pend) > 2:
            emit_pv(nc, pend.pop(0))

    for st in pend:
        emit_pv(nc, st)
    ctx.close()


# --------------------------------------------------------------------------
# host side
# --------------------------------------------------------------------------
def _layer_norm(x, g, b, eps=1e-5):
    m = x.mean(-1, keepdims=True)
    v = x.var(-1, keepdims=True)
    return (x - m) / np.sqrt(v + eps) * g + b


def compute_bias(rpe_biases, rel_index, pos_proj_w, pos_proj_b, ln1_g, ln1_b,
                 fc1_w, fc1_b, ln2_g, ln2_b, fc2_w, fc2_b, ln3_g, ln3_b,
                 fc3_w, fc3_b):
    """pos-bias MLP + gather, in fp64 on host -> (6, 256, 256) [h, q, k]."""
    f8 = np.float64
    p = rpe_biases.astype(f8) @ pos_proj_w.astype(f8) + pos_proj_b.astype(f8)
    p = np.maximum(_layer_norm(p, ln1_g.astype(f8), ln1_b.astype(f8)), 0)
    p = p @ fc1_w.astype(f8) + fc1_b.astype(f8)
    p = np.maximum(_layer_norm(p, ln2_g.astype(f8), ln2_b.astype(f8)), 0)
    p = p @ fc2_w.astype(f8) + fc2_b.astype(f8)
    p = np.maximum(_layer_norm(p, ln3_g.astype(f8), ln3_b.astype(f8)), 0)
    pos = p @ fc3_w.astype(f8) + fc3_b.astype(f8)          # (num_biases, 6)
    rel = pos[np.asarray(rel_index).reshape(-1)]
    return np.ascontiguousarray(
        rel.reshape(N, N, NUM_HEADS).transpose(2, 0, 1)).astype(np.float64)


def im2win(x):
    """(B, L, C) -> (512, 256, C) windows in (b, hb, wb) / (hs, ws) order."""
    x = x.reshape(B, H // H_SP, H_SP, W // W_SP, W_SP, DIM)
    x = x.transpose(0, 1, 3, 2, 4, 5)
    return np.ascontiguousarray(x.reshape(NW_TOTAL, N, DIM))


def bias_banks(bias):
    """(6,256,256) fp64 [h,q,k] -> per-head [128, 512] LAM-scaled bank
    layout: col = 256*kc + q, partition = k_local."""
    bl = LAM * bias                                       # (6, 256, 256)
    bt = bl.transpose(0, 2, 1).reshape(NUM_HEADS, 2, 128, N)  # h, kc, k, q
    return np.ascontiguousarray(bt.transpose(0, 2, 1, 3)).reshape(
        NUM_HEADS, 128, 512)                               # fp64


def prep_inputs(qkv, bias):
    import ml_dtypes
    e4m3 = ml_dtypes.float8_e4m3

    q = im2win(np.asarray(qkv[0]))
    k = im2win(np.asarray(qkv[1]))
    v = im2win(np.asarray(qkv[2]))

    qTf = np.ascontiguousarray(
        (q * np.float32(SCALE * LAM)).transpose(2, 0, 1)).astype(np.float16)
    kTf = np.ascontiguousarray(k.transpose(2, 0, 1)).astype(np.float16)

    vr = v.reshape(NW_TOTAL, 2, 128, NUM_HEADS, HEAD_DIM)
    ones = np.ones((NW_TOTAL, 2, 128, NUM_HEADS, 1), np.float32)
    vAf = np.concatenate([vr, ones], -1)
    vAf = np.ascontiguousarray(
        vAf.reshape(NW_TOTAL, 2, 128, 198).transpose(2, 0, 1, 3)
    ).reshape(128, NW_TOTAL, 396).astype(np.float16)

    bb = bias_banks(bias)                                  # (6, 128, 512)
    # fp8 coarse+residual DoubleRow moving operands for the DR-covered
    # prefix of each group (banks 0,1 fully; bank 2 up to DR_COLS-1024),
    # and the fp16 in1 bias for the DVE tail (zeros where DR covered).
    drs, bls = [], []
    for grp in range(2):
        heads = (0, 1, 2) if grp == 0 else (3, 4, 5)
        drs.append(np.concatenate(
            [bb[heads[0]], bb[heads[1]],
             bb[heads[2]][:, :DR_COLS - 1024]], axis=1))
        tail = bb[heads[2]][:, ACT_COLS - 1024:].copy()    # [128, STT_COLS]
        tail[:, :DR_COLS - ACT_COLS] = 0.0
        bls.append(tail)
    biasDR = np.concatenate(drs, axis=1).astype(np.float16)
    biasL = np.concatenate(bls, axis=1).astype(np.float16)
    id2 = np.eye(128, dtype=np.float32).astype(np.float16)
    return qTf, kTf, vAf, biasDR, biasL, id2


def _run(qkv, rpe_biases, rel_index, params, trace=False, **spmd_kwargs):
    qkv = np.asarray(qkv, np.float32)
    bias = compute_bias(np.asarray(rpe_biases), np.asarray(rel_index), **params)
    qTf, kTf, vAf, biasDR, biasL, id2 = prep_inputs(qkv, bias)

    nc = build_program(NW)
    in_maps = []
    for c in range(N_CORES):
        s = slice(c * NW, (c + 1) * NW)
        in_maps.append({
            "qT": np.ascontiguousarray(qTf[:, s]).reshape(DIM, NW * N),
            "kT": np.ascontiguousarray(kTf[:, s]).reshape(DIM, NW * N),
            "vA": np.ascontiguousarray(vAf[:, s]).reshape(128, NW * 396),
            "biasDR": biasDR, "biasL": biasL, "id2": id2,
        })
    res = run_bass_kernel_spmd(nc, in_maps, core_ids=list(range(N_CORES)),
                               trace=trace, **spmd_kwargs)

    outw = np.stack([np.asarray(res.results[c]["outw"], np.float32)
                     for c in range(N_CORES)])
    x = outw.reshape(N_CORES, 128, NW, 2, DIM).transpose(0, 2, 3, 1, 4)
    return unwindow(x.reshape(NW_TOTAL, N, DIM)), res


def kernel(qkv, H=None, W=None, rpe_biases=None, rel_index=None, **params):
    return _run(qkv, rpe_biases, rel_index, params)[0]


def unwindow(x):
    """(512, 256, 192) -> (B, H, W, C)"""
    x = x.reshape(B, H // H_SP, W // W_SP, H_SP, W_SP, DIM)
    x = x.transpose(0, 1, 3, 2, 4, 5)
    return np.ascontiguousarray(x.reshape(B, H, W, DIM))


# revision 29
# speedup vs baseline: 1.1952x; 1.1952x over previous
"""Trainium2 Bass kernel for windowed multi-head attention with dynamic
position bias (sparse_attention, B=2, H=W=256, 8x32 windows, 6 heads, d=32).

v2: three-engine softmax. The baseline was simultaneously ScalarE-bound
(exp of all 6x256x256 logits/window, 201us busy) and PE-bound (bias
identity-matmuls = 44% of PE streaming). This version:
  - replaces the fp16 bias identity-matmuls with fp8 DoubleRow matmuls
    (identity|identity stationary, coarse|residual moving) at 0.5
    cycles/column -- half the PE streaming cost, ~3e-4 bias error.
  - splits the exp work: ScalarE does true exp for heads {0,1} of each
    3-head PSUM group (2048 cols/window); head 2 (1024 cols/window) uses
    a Schraudolph-style bit-trick: Pool adds a magic constant to the PSUM
    logits (pre-scaled by 1024*log2e via Q) and converts to int16 = the
    fp16 bit pattern of 2^i(1+f); one custom DVE uop-pipeline op applies
    a minimax quadratic mantissa correction (rel err <= 5.4e-3 on that
    1/3 slice, exact elsewhere) straight into the fp16 P tile.
  - PV + reciprocal-normalize unchanged in spirit; output fp16.
"""

import sys

sys.path.insert(0, "/opt/trn_rl_repo")

import numpy as np

import concourse.bass as bass
import concourse.tile as tile
from concourse import mybir
from concourse.alu_op_type import AluOpType
from concourse.bass_utils import run_bass_kernel_spmd

F32 = mybir.dt.float32
BF16 = mybir.dt.bfloat16
FP16 = mybir.dt.float16
FP8 = mybir.dt.float8e4
I16 = mybir.dt.int16
EXP = mybir.ActivationFunctionType.Exp

N_CORES = 8
B, H, W = 2, 256, 256
H_SP, W_SP = 8, 32
NUM_HEADS = 6
DIM = 192
HEAD_DIM = 32
SCALE = HEAD_DIM ** -0.5
N = H_SP * W_SP                     # 256 tokens / window
NW_TOTAL = B * (H // H_SP) * (W // W_SP)   # 512 windows
NW = NW_TOTAL // N_CORES            # 64 windows / core

# Schraudolph exp2 path: PSUM holds t = LAM*(s+bias); u = int16(t + MAGIC)
# bitcast fp16 is 2^(i-15)(1+f); P = u * ((m + PC1)*m + PC2) with m = 1+f
# extracted from the fp32 bit pattern ((bits|0x3F800000)&0x3FFFFFFF).
LAM = 1024.0 * np.log2(np.e)        # 1477.3195458...
MAGIC = 13205.898538354311
PC1 = -2.9537455388278904
PC2 = 6.228467047720157
MASK_CONST = float(np.int32(0x3FFFFFFF).view(np.float32))
ACT_SCALE = float(np.log(2.0) / 1024.0)

# per 3-head PSUM group [128, 1536]: cols [0,ACT_COLS) get ScalarE true
# exp; the rest go through the DVE int16 bit-trick (GPSIMD cannot touch
# PSUM on trn2, so both legs of the split are ScalarE/DVE). The fp8
# DoubleRow bias matmuls only cover [0,DR_COLS) -- the S matmuls there
# accumulate (start=False); past it they start fresh and the bias rides
# the DVE tensor_scalar's in1 (zero-padded over [ACT_COLS,DR_COLS)).
ACT_COLS = 1248
STT_COLS = 1536 - ACT_COLS
DR_COLS = 1280                      # banks 0,1 fully + bank 2 kc0 region


# --------------------------------------------------------------------------
# custom DVE op: out = in0 * ((m + s1)*m + imm2), m = 1+frac(mantissa(in0))
# --------------------------------------------------------------------------
def _register_custom_ops():
    from concourse import dve_ops as DO
    from concourse.dve_spec import (Spec, Src0, Src1, C0, C1, C2, One, Bin,
                                    lower, _has_src1)
    from concourse.dve_uop import AluOp, DveOpSpec

    def _reg(name, spec):
        if name in DO._SUB_OPCODE_FOR_NAME:
            return next(op for op in DO.OPS if op.name == name)
        row = DO._CUSTOM_DVE_ROW_BASE + len(DO.OPS)
        sha = DveOpSpec(name=name, opcode=row, uops=lower(spec, ver="v3"),
                        rd1_en=_has_src1(spec)).sha("v3")
        op = DO.DveOp(name, spec, subdim=False, uops_sha={"v3": sha})
        DO.OPS.append(op)
        DO._SUB_OPCODE_FOR_NAME[name] = row
        DO.CUSTOM_DVE_SPECS[name] = spec
        return op

    # exp2 mantissa fixup: out = in0 * ((m + s1)*m + imm2),
    # m = 1+frac extracted from the fp32 bit pattern of in0.
    _m = Bin(AluOp.BITWISE_AND, Bin(AluOp.BITWISE_OR, Src0, One), C0)

    def _fix_ref(in0, in1, s0, s1, imm2):
        x = np.ascontiguousarray(in0.astype(np.float32))
        bits = x.view(np.int32)
        mm = ((bits | 0x3F800000) & 0x3FFFFFFF).view(np.float32)
        return (x * ((mm + s1) * mm + imm2)).astype(np.float32)

    fixup = _reg("EXP2_FIXUP_ANT",
                 Spec(body=Src0 * ((_m + C1) * _m + C2), reference=_fix_ref))

    # fused normalize: out = in0 * recip_approx(in1); BITWISE_NOT seed +
    # one Newton step (~0.4% max rel err, cancels nothing downstream).
    _ny = Bin(AluOp.BITWISE_NOT, Src1, Src1)
    _y0 = _ny * C0
    _y1 = _y0 * (C1 - Src1 * _y0)

    def _nrm_ref(in0, in1, s0, s1, imm2):
        x1 = np.ascontiguousarray(in1.astype(np.float32))
        ny = (~x1.view(np.int32)).view(np.float32)
        y0 = ny * np.float32(s0)
        y1 = y0 * (np.float32(s1) - x1 * y0)
        return (in0.astype(np.float32) * y1).astype(np.float32)

    nrm = _reg("NORM_RECIP_ANT", Spec(body=Src0 * _y1, reference=_nrm_ref))
    return fixup, nrm


EXP2_FIXUP, NORM_RECIP = _register_custom_ops()
# Chebyshev seed constants for x*bitcast(~x) in [-4.5, -4] (see dve_ops)
NR_C0 = -0.23549792
NR_C1 = 2.0017324


# --------------------------------------------------------------------------
# device program
# --------------------------------------------------------------------------
WG = 8     # windows per input slab
OG = 4     # windows per output slab


def build_program(nw=NW):
    from concourse import bacc
    nc = bacc.Bacc("TRN2", target_bir_lowering=False, debug=False)

    qT = nc.dram_tensor("qT", [DIM, nw * N], FP16, kind="ExternalInput").ap()
    kT = nc.dram_tensor("kT", [DIM, nw * N], FP16, kind="ExternalInput").ap()
    vA = nc.dram_tensor("vA", [128, nw * 396], FP16, kind="ExternalInput").ap()
    biasDR = nc.dram_tensor("biasDR", [128, 2 * DR_COLS], FP16,
                            kind="ExternalInput").ap()
    biasL = nc.dram_tensor("biasL", [128, 2 * STT_COLS], FP16,
                           kind="ExternalInput").ap()
    id2 = nc.dram_tensor("id2", [128, 128], FP16, kind="ExternalInput").ap()
    outw = nc.dram_tensor("outw", [128, nw * 2 * DIM], FP16,
                          kind="ExternalOutput").ap()

    with tile.TileContext(nc) as tc:
        _emit(nc, tc, nw, qT, kT, vA, biasDR, biasL, id2, outw)
    nc.compile()
    return nc


def _emit(nc, tc, nw, qT, kT, vA, biasDR, biasL, id2, outw):
    from contextlib import ExitStack
    ctx = ExitStack()

    # residents: fp8 DoubleRow bias (coarse|residual per covered bank
    # range, LAM-scaled), the doubled identity stationary, and the
    # zero-padded fp16 bias for the DVE slice.
    bdr_sb = nc.alloc_sbuf_tensor("bdr_sb", [128, 2 * DR_COLS], FP16).ap()
    bl_sb = nc.alloc_sbuf_tensor("bl_sb", [128, 2 * STT_COLS], FP16).ap()
    id2_sb = nc.alloc_sbuf_tensor("id2_sb", [128, 128], FP16).ap()
    nc.sync.dma_start(bdr_sb, biasDR)
    nc.sync.dma_start(bl_sb, biasL)
    nc.sync.dma_start(id2_sb, id2)

    pin = ctx.enter_context(tc.tile_pool(name="pin", bufs=2))
    pps = ctx.enter_context(tc.tile_pool(name="pps", bufs=2, space="PSUM"))
    ppt = ctx.enter_context(tc.tile_pool(name="ppt", bufs=2))
    pu = ctx.enter_context(tc.tile_pool(name="pu", bufs=2))
    pout = ctx.enter_context(tc.tile_pool(name="pout", bufs=4))

    qa = qb = ka = kb = va = None
    obh = [None]
    pend = []        # (pt, va, wv, w) queue; PV runs two windows behind

    def emit_pv(nc, state):
        pt, pva, pwv, pw = state
        if pw % OG == 0:
            obn = pout.tile([128, OG * 2 * DIM], FP16, tag="ob", bufs=3)
            obh[0] = obn
        ob = obh[0]
        pv = pps.tile([128, 396], F32, tag="pv", bufs=2)
        for qc in (0, 1):
            for h in range(NUM_HEADS):
                base = 1536 * (h // 3) + 512 * (h % 3)
                for kc in (0, 1):
                    nc.tensor.matmul(
                        pv[:, 198 * qc + 33 * h: 198 * qc + 33 * h + 33],
                        lhsT=pt[:, base + 256 * kc + 128 * qc:
                                base + 256 * kc + 128 * qc + 128],
                        rhs=pva[:, pwv + 198 * kc + 33 * h:
                                pwv + 198 * kc + 33 * h + 33],
                        start=(kc == 0), stop=(kc == 1),
                        skip_group_check=True,
                    )
        # normalize: ob = pv * recip(rowsum), rowsum in col 32
        pv3 = pv.rearrange("p (g c) -> p g c", c=33)
        rv = pout.tile([128, 16], F32, tag="rv", bufs=4)
        nc.vector.reciprocal_approx_fast(rv[:, 0:12], pv3[:, :, 32])
        oslot = ob[:, (pw % OG) * 2 * DIM: (pw % OG) * 2 * DIM + 2 * DIM]
        nc.vector.tensor_tensor(
            oslot.rearrange("p (g c) -> p g c", c=32),
            pv3[:, :, 0:32],
            rv[:, 0:12].unsqueeze(-1).broadcast_to([128, 12, 32]),
            op=AluOpType.mult,
        )
        if pw == nw - 3 and nw >= 8 and OG == 4:   # early half-flush
            base = (pw - 1) * 2 * DIM
            nc.sync.dma_start(outw[:, base: base + 2 * 2 * DIM],
                              ob[:, 0: 2 * 2 * DIM])
        elif pw == nw - 1 and nw >= 8 and OG == 4:
            base = (pw - 1) * 2 * DIM
            nc.sync.dma_start(outw[:, base: base + 2 * 2 * DIM],
                              ob[:, 2 * 2 * DIM: 4 * 2 * DIM])
        elif pw % OG == OG - 1:
            base = (pw - (OG - 1)) * 2 * DIM
            nc.sync.dma_start(outw[:, base: base + OG * 2 * DIM], ob)

    slabs = [(0, 1), (1, 1), (2, 2), (4, 4)] + \
        [(s, WG) for s in range(WG, nw, WG)]
    slab_of = {}
    for s0, sn in slabs:
        for i in range(sn):
            slab_of[s0 + i] = s0

    for w in range(nw):
        if slab_of[w] == w:
            sn = dict(slabs)[w]
            g = w * N
            qa = pin.tile([128, WG * N], FP16, tag="qa",
                          padded_shape=[128, WG * N])
            nc.sync.dma_start(qa[:, 0:sn * N], qT[0:128, g:g + sn * N])
            qb = pin.tile([64, WG * N], FP16, tag="qb",
                          padded_shape=[64, WG * N])
            nc.sync.dma_start(qb[:, 0:sn * N], qT[128:192, g:g + sn * N])
            ka = pin.tile([128, WG * N], FP16, tag="ka",
                          padded_shape=[128, WG * N])
            nc.sync.dma_start(ka[:, 0:sn * N], kT[0:128, g:g + sn * N])
            kb = pin.tile([64, WG * N], FP16, tag="kb",
                          padded_shape=[64, WG * N])
            nc.sync.dma_start(kb[:, 0:sn * N], kT[128:192, g:g + sn * N])
            va = pin.tile([128, WG * 396], FP16, tag="va",
                          padded_shape=[128, WG * 396])
            nc.sync.dma_start(va[:, 0:sn * 396],
                              vA[:, w * 396:(w + sn) * 396])
        wq = (w - slab_of[w]) * N
        wv = (w - slab_of[w]) * 396

        pt = ppt.tile([128, 3072], FP16, tag="pt", bufs=3)
        u16 = pu.tile([128, 2 * STT_COLS], I16, tag="u16")
        uf16 = u16.bitcast(FP16)

        for grp in range(2):
            s = pps.tile([128, 1536], F32, tag="s")
            heads = (0, 1, 2) if grp == 0 else (3, 4, 5)

            # bias lands first via fp8 DoubleRow identity matmuls (coarse
            # + residual halves) over [0, DR_COLS), then K'Q accumulates;
            # past DR_COLS the S matmul starts fresh.
            dr_off = DR_COLS * grp
            for hh, cov in ((0, 512), (1, 512), (2, DR_COLS - 1024)):
                nc.tensor.matmul(
                    s[:, 512 * hh: 512 * hh + cov],
                    lhsT=id2_sb,
                    rhs=bdr_sb[:, dr_off + 512 * hh:
                               dr_off + 512 * hh + cov],
                    start=True, stop=False, skip_group_check=True,
                )
            for kc in (0, 1):
                for h in heads:
                    hh = h % 3
                    hp = h if h < 4 else h - 4
                    ktile = ka if h < 4 else kb
                    qtile = qa if h < 4 else qb
                    nc.tensor.matmul(
                        s[:, 512 * hh + 256 * kc: 512 * hh + 256 * kc + 256],
                        lhsT=ktile[32 * hp: 32 * hp + 32,
                                   wq + 128 * kc: wq + 128 * kc + 128],
                        rhs=qtile[32 * hp: 32 * hp + 32, wq: wq + N],
                        start=(512 * hh + 256 * kc >= DR_COLS),
                        stop=(kc == 1),
                        tile_position=(32 * hp, 0), skip_group_check=True,
                    )

            # exp: ScalarE true exp on the DR-biased prefix; DVE bit-trick
            # on the rest (add magic + in1 bias tail, convert to int16,
            # then the custom DVE op applies the mantissa fixup into pt).
            nc.scalar.activation(pt[:, 1536 * grp: 1536 * grp + ACT_COLS],
                                 s[:, 0:ACT_COLS], EXP, scale=ACT_SCALE)
            nc.vector.scalar_tensor_tensor(
                u16[:, STT_COLS * grp: STT_COLS * grp + STT_COLS],
                s[:, ACT_COLS:1536], MAGIC,
                bl_sb[:, STT_COLS * grp: STT_COLS * grp + STT_COLS],
                op0=AluOpType.add, op1=AluOpType.add,
            )
            nc.vector._custom_dve(
                EXP2_FIXUP,
                out=pt[:, 1536 * grp + ACT_COLS: 1536 * grp + 1536],
                in0=uf16[:, STT_COLS * grp: STT_COLS * grp + STT_COLS],
                s0=MASK_CONST, s1=PC1, imm2=PC2,
            )

        pend.append((pt, va, wv, w))
        if len(pend) > 2:
            emit_pv(nc, pend.pop(0))

    for st in pend:
        emit_pv(nc, st)
    ctx.close()


# --------------------------------------------------------------------------
# host side
# --------------------------------------------------------------------------
def _layer_norm(x, g, b, eps=1e-5):
    m = x.mean(-1, keepdims=True)
    v = x.var(-1, keepdims=True)
    return (x - m) / np.sqrt(v + eps) * g + b


def compute_bias(rpe_biases, rel_index, pos_proj_w, pos_proj_b, ln1_g, ln1_b,
                 fc1_w, fc1_b, ln2_g, ln2_b, fc2_w, fc2_b, ln3_g, ln3_b,
                 fc3_w, fc3_b):
    """pos-bias MLP + gather, in fp64 on host -> (6, 256, 256) [h, q, k]."""
    f8 = np.float64
    p = rpe_biases.astype(f8) @ pos_proj_w.astype(f8) + pos_proj_b.astype(f8)
    p = np.maximum(_layer_norm(p, ln1_g.astype(f8), ln1_b.astype(f8)), 0)
    p = p @ fc1_w.astype(f8) + fc1_b.astype(f8)
    p = np.maximum(_layer_norm(p, ln2_g.astype(f8), ln2_b.astype(f8)), 0)
    p = p @ fc2_w.astype(f8) + fc2_b.astype(f8)
    p = np.maximum(_layer_norm(p, ln3_g.astype(f8), ln3_b.astype(f8)), 0)
    pos = p @ fc3_w.astype(f8) + fc3_b.astype(f8)          # (num_biases, 6)
    rel = pos[np.asarray(rel_index).reshape(-1)]
    return np.ascontiguousarray(
        rel.reshape(N, N, NUM_HEADS).transpose(2, 0, 1)).astype(np.float64)


def im2win(x):
    """(B, L, C) -> (512, 256, C) windows in (b, hb, wb) / (hs, ws) order."""
    x = x.reshape(B, H // H_SP, H_SP, W // W_SP, W_SP, DIM)
    x = x.transpose(0, 1, 3, 2, 4, 5)
    return np.ascontiguousarray(x.reshape(NW_TOTAL, N, DIM))


def bias_banks(bias):
    """(6,256,256) fp64 [h,q,k] -> per-head [128, 512] LAM-scaled bank
    layout: col = 256*kc + q, partition = k_local."""
    bl = LAM * bias                                       # (6, 256, 256)
    bt = bl.transpose(0, 2, 1).reshape(NUM_HEADS, 2, 128, N)  # h, kc, k, q
    return np.ascontiguousarray(bt.transpose(0, 2, 1, 3)).reshape(
        NUM_HEADS, 128, 512)                               # fp64


def prep_inputs(qkv, bias):
    import ml_dtypes
    e4m3 = ml_dtypes.float8_e4m3

    q = im2win(np.asarray(qkv[0]))
    k = im2win(np.asarray(qkv[1]))
    v = im2win(np.asarray(qkv[2]))

    qTf = np.ascontiguousarray(
        (q * np.float32(SCALE * LAM)).transpose(2, 0, 1)).astype(np.float16)
    kTf = np.ascontiguousarray(k.transpose(2, 0, 1)).astype(np.float16)

    vr = v.reshape(NW_TOTAL, 2, 128, NUM_HEADS, HEAD_DIM)
    ones = np.ones((NW_TOTAL, 2, 128, NUM_HEADS, 1), np.float32)
    vAf = np.concatenate([vr, ones], -1)
    vAf = np.ascontiguousarray(
        vAf.reshape(NW_TOTAL, 2, 128, 198).transpose(2, 0, 1, 3)
    ).reshape(128, NW_TOTAL, 396).astype(np.float16)

    bb = bias_banks(bias)                                  # (6, 128, 512)
    # fp8 coarse+residual DoubleRow moving operands for the DR-covered
    # prefix of each group (banks 0,1 fully; bank 2 up to DR_COLS-1024),
    # and the fp16 in1 bias for the DVE tail (zeros where DR covered).
    drs, bls = [], []
    for grp in range(2):
        heads = (0, 1, 2) if grp == 0 else (3, 4, 5)
        drs.append(np.concatenate(
            [bb[heads[0]], bb[heads[1]],
             bb[heads[2]][:, :DR_COLS - 1024]], axis=1))
        tail = bb[heads[2]][:, ACT_COLS - 1024:].copy()    # [128, STT_COLS]
        tail[:, :DR_COLS - ACT_COLS] = 0.0
        bls.append(tail)
    biasDR = np.concatenate(drs, axis=1).astype(np.float16)
    biasL = np.concatenate(bls, axis=1).astype(np.float16)
    id2 = np.eye(128, dtype=np.float32).astype(np.float16)
    return qTf, kTf, vAf, biasDR, biasL, id2


def _run(qkv, rpe_biases, rel_index, params, trace=False, **spmd_kwargs):
    qkv = np.asarray(qkv, np.float32)
    bias = compute_bias(np.asarray(rpe_biases), np.asarray(rel_index), **params)
    qTf, kTf, vAf, biasDR, biasL, id2 = prep_inputs(qkv, bias)

    nc = build_program(NW)
    in_maps = []
    for c in range(N_CORES):
        s = slice(c * NW, (c + 1) * NW)
        in_maps.append({
            "qT": np.ascontiguousarray(qTf[:, s]).reshape(DIM, NW * N),
            "kT": np.ascontiguousarray(kTf[:, s]).reshape(DIM, NW * N),
            "vA": np.ascontiguousarray(vAf[:, s]).reshape(128, NW * 396),
            "biasDR": biasDR, "biasL": biasL, "id2": id2,
        })
    res = run_bass_kernel_spmd(nc, in_maps, core_ids=list(range(N_CORES)),
                               trace=trace, **spmd_kwargs)

    outw = np.stack([np.asarray(res.results[c]["outw"], np.float32)
                     for c in range(N_CORES)])
    x = outw.reshape(N_CORES, 128, NW, 2, DIM).transpose(0, 2, 3, 1, 4)
    return unwindow(x.reshape(NW_TOTAL, N, DIM)), res


def kernel(qkv, H=None, W=None, rpe_biases=None, rel_index=None, **params):
    return _run(qkv, rpe_biases, rel_index, params)[0]


def unwindow(x):
    """(512, 256, 192) -> (B, H, W, C)"""
    x = x.reshape(B, H // H_SP, W // W_SP, H_SP, W_SP, DIM)
    x = x.transpose(0, 1, 3, 2, 4, 5)
    return np.ascontiguousarray(x.reshape(B, H, W, DIM))


# revision 31
# speedup vs baseline: 1.1987x; 1.0029x over previous
"""Trainium2 Bass kernel for windowed multi-head attention with dynamic
position bias (sparse_attention, B=2, H=W=256, 8x32 windows, 6 heads, d=32).

v2: three-engine softmax. The baseline was simultaneously ScalarE-bound
(exp of all 6x256x256 logits/window, 201us busy) and PE-bound (bias
identity-matmuls = 44% of PE streaming). This version:
  - replaces the fp16 bias identity-matmuls with fp8 DoubleRow matmuls
    (identity|identity stationary, coarse|residual moving) at 0.5
    cycles/column -- half the PE streaming cost, ~3e-4 bias error.
  - splits the exp work: ScalarE does true exp for heads {0,1} of each
    3-head PSUM group (2048 cols/window); head 2 (1024 cols/window) uses
    a Schraudolph-style bit-trick: Pool adds a magic constant to the PSUM
    logits (pre-scaled by 1024*log2e via Q) and converts to int16 = the
    fp16 bit pattern of 2^i(1+f); one custom DVE uop-pipeline op applies
    a minimax quadratic mantissa correction (rel err <= 5.4e-3 on that
    1/3 slice, exact elsewhere) straight into the fp16 P tile.
  - PV + reciprocal-normalize unchanged in spirit; output fp16.
"""

import sys

sys.path.insert(0, "/opt/trn_rl_repo")

import numpy as np

import concourse.bass as bass
import concourse.tile as tile
from concourse import mybir
from concourse.alu_op_type import AluOpType
from concourse.bass_utils import run_bass_kernel_spmd

F32 = mybir.dt.float32
BF16 = mybir.dt.bfloat16
FP16 = mybir.dt.float16
FP8 = mybir.dt.float8e4
I16 = mybir.dt.int16
EXP = mybir.ActivationFunctionType.Exp

N_CORES = 8
B, H, W = 2, 256, 256
H_SP, W_SP = 8, 32
NUM_HEADS = 6
DIM = 192
HEAD_DIM = 32
SCALE = HEAD_DIM ** -0.5
N = H_SP * W_SP                     # 256 tokens / window
NW_TOTAL = B * (H // H_SP) * (W // W_SP)   # 512 windows
NW = NW_TOTAL // N_CORES            # 64 windows / core

# Schraudolph exp2 path: PSUM holds t = LAM*(s+bias); u = int16(t + MAGIC)
# bitcast fp16 is 2^(i-15)(1+f); P = u * ((m + PC1)*m + PC2) with m = 1+f
# extracted from the fp32 bit pattern ((bits|0x3F800000)&0x3FFFFFFF).
LAM = 1024.0 * np.log2(np.e)        # 1477.3195458...
MAGIC = 13205.898538354311
PC1 = -2.9537455388278904
PC2 = 6.228467047720157
MASK_CONST = float(np.int32(0x3FFFFFFF).view(np.float32))
ACT_SCALE = float(np.log(2.0) / 1024.0)

# per 3-head PSUM group [128, 1536]: cols [0,ACT_COLS) get ScalarE true
# exp; the rest go through the DVE int16 bit-trick (GPSIMD cannot touch
# PSUM on trn2, so both legs of the split are ScalarE/DVE). The fp8
# DoubleRow bias matmuls only cover [0,DR_COLS) -- the S matmuls there
# accumulate (start=False); past it they start fresh and the bias rides
# the DVE tensor_scalar's in1 (zero-padded over [ACT_COLS,DR_COLS)).
ACT_COLS = 1248
STT_COLS = 1536 - ACT_COLS
DR_COLS = 1280                      # banks 0,1 fully + bank 2 kc0 region


# --------------------------------------------------------------------------
# custom DVE op: out = in0 * ((m + s1)*m + imm2), m = 1+frac(mantissa(in0))
# --------------------------------------------------------------------------
def _register_custom_ops():
    from concourse import dve_ops as DO
    from concourse.dve_spec import (Spec, Src0, Src1, C0, C1, C2, One, Bin,
                                    lower, _has_src1)
    from concourse.dve_uop import AluOp, DveOpSpec

    def _reg(name, spec):
        if name in DO._SUB_OPCODE_FOR_NAME:
            return next(op for op in DO.OPS if op.name == name)
        row = DO._CUSTOM_DVE_ROW_BASE + len(DO.OPS)
        sha = DveOpSpec(name=name, opcode=row, uops=lower(spec, ver="v3"),
                        rd1_en=_has_src1(spec)).sha("v3")
        op = DO.DveOp(name, spec, subdim=False, uops_sha={"v3": sha})
        DO.OPS.append(op)
        DO._SUB_OPCODE_FOR_NAME[name] = row
        DO.CUSTOM_DVE_SPECS[name] = spec
        return op

    # exp2 mantissa fixup: out = in0 * ((m + s1)*m + imm2),
    # m = 1+frac extracted from the fp32 bit pattern of in0.
    _m = Bin(AluOp.BITWISE_AND, Bin(AluOp.BITWISE_OR, Src0, One), C0)

    def _fix_ref(in0, in1, s0, s1, imm2):
        x = np.ascontiguousarray(in0.astype(np.float32))
        bits = x.view(np.int32)
        mm = ((bits | 0x3F800000) & 0x3FFFFFFF).view(np.float32)
        return (x * ((mm + s1) * mm + imm2)).astype(np.float32)

    fixup = _reg("EXP2_FIXUP_ANT",
                 Spec(body=Src0 * ((_m + C1) * _m + C2), reference=_fix_ref))

    # fused normalize: out = in0 * recip_approx(in1); BITWISE_NOT seed +
    # one Newton step (~0.4% max rel err, cancels nothing downstream).
    _ny = Bin(AluOp.BITWISE_NOT, Src1, Src1)
    _y0 = _ny * C0
    _y1 = _y0 * (C1 - Src1 * _y0)

    def _nrm_ref(in0, in1, s0, s1, imm2):
        x1 = np.ascontiguousarray(in1.astype(np.float32))
        ny = (~x1.view(np.int32)).view(np.float32)
        y0 = ny * np.float32(s0)
        y1 = y0 * (np.float32(s1) - x1 * y0)
        return (in0.astype(np.float32) * y1).astype(np.float32)

    nrm = _reg("NORM_RECIP_ANT", Spec(body=Src0 * _y1, reference=_nrm_ref))
    return fixup, nrm


EXP2_FIXUP, NORM_RECIP = _register_custom_ops()
# Chebyshev seed constants for x*bitcast(~x) in [-4.5, -4] (see dve_ops)
NR_C0 = -0.23549792
NR_C1 = 2.0017324


# --------------------------------------------------------------------------
# device program
# --------------------------------------------------------------------------
WG = 8     # windows per input slab
OG = 4     # windows per output slab


def build_program(nw=NW):
    from concourse import bacc
    nc = bacc.Bacc("TRN2", target_bir_lowering=False, debug=False)

    qT = nc.dram_tensor("qT", [DIM, nw * N], FP16, kind="ExternalInput").ap()
    kT = nc.dram_tensor("kT", [DIM, nw * N], FP16, kind="ExternalInput").ap()
    vA = nc.dram_tensor("vA", [128, nw * 396], FP16, kind="ExternalInput").ap()
    biasDR = nc.dram_tensor("biasDR", [128, 2 * DR_COLS], FP16,
                            kind="ExternalInput").ap()
    biasL = nc.dram_tensor("biasL", [128, 2 * STT_COLS], FP16,
                           kind="ExternalInput").ap()
    id2 = nc.dram_tensor("id2", [128, 128], FP16, kind="ExternalInput").ap()
    outw = nc.dram_tensor("outw", [128, nw * 2 * DIM], FP16,
                          kind="ExternalOutput").ap()

    with tile.TileContext(nc) as tc:
        _emit(nc, tc, nw, qT, kT, vA, biasDR, biasL, id2, outw)
    nc.compile()
    return nc


def _emit(nc, tc, nw, qT, kT, vA, biasDR, biasL, id2, outw):
    from contextlib import ExitStack
    ctx = ExitStack()

    # residents: fp8 DoubleRow bias (coarse|residual per covered bank
    # range, LAM-scaled), the doubled identity stationary, and the
    # zero-padded fp16 bias for the DVE slice.
    bdr_sb = nc.alloc_sbuf_tensor("bdr_sb", [128, 2 * DR_COLS], FP16).ap()
    bl_sb = nc.alloc_sbuf_tensor("bl_sb", [128, 2 * STT_COLS], FP16).ap()
    id2_sb = nc.alloc_sbuf_tensor("id2_sb", [128, 128], FP16).ap()
    nc.sync.dma_start(bdr_sb, biasDR)
    nc.sync.dma_start(bl_sb, biasL)
    nc.sync.dma_start(id2_sb, id2)

    pin = ctx.enter_context(tc.tile_pool(name="pin", bufs=2))
    pps = ctx.enter_context(tc.tile_pool(name="pps", bufs=2, space="PSUM"))
    ppt = ctx.enter_context(tc.tile_pool(name="ppt", bufs=2))
    pu = ctx.enter_context(tc.tile_pool(name="pu", bufs=2))
    pout = ctx.enter_context(tc.tile_pool(name="pout", bufs=4))

    qa = qb = ka = kb = va = None
    obh = [None]
    pend = []        # (pt, va, wv, w) queue; PV runs two windows behind

    def emit_pv(nc, state):
        pt, pva, pwv, pw = state
        if pw % OG == 0:
            obn = pout.tile([128, OG * 2 * DIM], FP16, tag="ob", bufs=3)
            obh[0] = obn
        ob = obh[0]
        pv = pps.tile([128, 396], F32, tag="pv", bufs=2)
        for qc in (0, 1):
            for h in range(NUM_HEADS):
                base = 1536 * (h // 3) + 512 * (h % 3)
                for kc in (0, 1):
                    nc.tensor.matmul(
                        pv[:, 198 * qc + 33 * h: 198 * qc + 33 * h + 33],
                        lhsT=pt[:, base + 256 * kc + 128 * qc:
                                base + 256 * kc + 128 * qc + 128],
                        rhs=pva[:, pwv + 198 * kc + 33 * h:
                                pwv + 198 * kc + 33 * h + 33],
                        start=(kc == 0), stop=(kc == 1),
                        skip_group_check=True,
                    )
        # normalize: ob = pv * recip(rowsum), rowsum in col 32
        pv3 = pv.rearrange("p (g c) -> p g c", c=33)
        rv = pout.tile([128, 16], F32, tag="rv", bufs=4)
        nc.vector.reciprocal_approx_fast(rv[:, 0:12], pv3[:, :, 32])
        oslot = ob[:, (pw % OG) * 2 * DIM: (pw % OG) * 2 * DIM + 2 * DIM]
        nc.vector.tensor_tensor(
            oslot.rearrange("p (g c) -> p g c", c=32),
            pv3[:, :, 0:32],
            rv[:, 0:12].unsqueeze(-1).broadcast_to([128, 12, 32]),
            op=AluOpType.mult,
        )
        if pw == nw - 3 and nw >= 8 and OG == 4:   # early half-flush
            base = (pw - 1) * 2 * DIM
            nc.sync.dma_start(outw[:, base: base + 2 * 2 * DIM],
                              ob[:, 0: 2 * 2 * DIM])
        elif pw == nw - 1 and nw >= 8 and OG == 4:
            base = (pw - 1) * 2 * DIM
            nc.sync.dma_start(outw[:, base: base + 2 * 2 * DIM],
                              ob[:, 2 * 2 * DIM: 4 * 2 * DIM])
        elif pw % OG == OG - 1:
            base = (pw - (OG - 1)) * 2 * DIM
            nc.sync.dma_start(outw[:, base: base + OG * 2 * DIM], ob)

    slabs = [(0, 1), (1, 1), (2, 2), (4, 4)] + \
        [(s, WG) for s in range(WG, nw, WG)]
    slab_of = {}
    for s0, sn in slabs:
        for i in range(sn):
            slab_of[s0 + i] = s0

    for w in range(nw):
        if slab_of[w] == w:
            sn = dict(slabs)[w]
            g = w * N
            qa = pin.tile([128, WG * N], FP16, tag="qa",
                          padded_shape=[128, WG * N])
            nc.sync.dma_start(qa[:, 0:sn * N], qT[0:128, g:g + sn * N])
            qb = pin.tile([64, WG * N], FP16, tag="qb",
                          padded_shape=[64, WG * N])
            nc.sync.dma_start(qb[:, 0:sn * N], qT[128:192, g:g + sn * N])
            ka = pin.tile([128, WG * N], FP16, tag="ka",
                          padded_shape=[128, WG * N])
            nc.sync.dma_start(ka[:, 0:sn * N], kT[0:128, g:g + sn * N])
            kb = pin.tile([64, WG * N], FP16, tag="kb",
                          padded_shape=[64, WG * N])
            nc.sync.dma_start(kb[:, 0:sn * N], kT[128:192, g:g + sn * N])
            va = pin.tile([128, WG * 396], FP16, tag="va",
                          padded_shape=[128, WG * 396])
            nc.sync.dma_start(va[:, 0:sn * 396],
                              vA[:, w * 396:(w + sn) * 396])
        wq = (w - slab_of[w]) * N
        wv = (w - slab_of[w]) * 396

        pt = ppt.tile([128, 3072], FP16, tag="pt", bufs=3)
        u16 = pu.tile([128, 2 * STT_COLS], I16, tag="u16")
        uf16 = u16.bitcast(FP16)

        for grp in range(2):
            s = pps.tile([128, 1536], F32, tag="s")
            heads = (0, 1, 2) if grp == 0 else (3, 4, 5)

            # bias lands first via fp8 DoubleRow identity matmuls (coarse
            # + residual halves) over [0, DR_COLS), then K'Q accumulates;
            # past DR_COLS the S matmul starts fresh.
            dr_off = DR_COLS * grp
            for hh, cov in ((0, 512), (1, 512), (2, DR_COLS - 1024)):
                nc.tensor.matmul(
                    s[:, 512 * hh: 512 * hh + cov],
                    lhsT=id2_sb,
                    rhs=bdr_sb[:, dr_off + 512 * hh:
                               dr_off + 512 * hh + cov],
                    start=True, stop=False, skip_group_check=True,
                )
            for kc in (0, 1):
                for h in heads:
                    hh = h % 3
                    hp = h if h < 4 else h - 4
                    ktile = ka if h < 4 else kb
                    qtile = qa if h < 4 else qb
                    nc.tensor.matmul(
                        s[:, 512 * hh + 256 * kc: 512 * hh + 256 * kc + 256],
                        lhsT=ktile[32 * hp: 32 * hp + 32,
                                   wq + 128 * kc: wq + 128 * kc + 128],
                        rhs=qtile[32 * hp: 32 * hp + 32, wq: wq + N],
                        start=(512 * hh + 256 * kc >= DR_COLS),
                        stop=(kc == 1),
                        tile_position=(32 * hp, 0), skip_group_check=True,
                    )

            # exp: ScalarE true exp on the DR-biased prefix; DVE bit-trick
            # on the rest (add magic + in1 bias tail, convert to int16,
            # then the custom DVE op applies the mantissa fixup into pt).
            nc.scalar.activation(pt[:, 1536 * grp: 1536 * grp + ACT_COLS],
                                 s[:, 0:ACT_COLS], EXP, scale=ACT_SCALE)
            nc.vector.scalar_tensor_tensor(
                u16[:, STT_COLS * grp: STT_COLS * grp + STT_COLS],
                s[:, ACT_COLS:1536], MAGIC,
                bl_sb[:, STT_COLS * grp: STT_COLS * grp + STT_COLS],
                op0=AluOpType.add, op1=AluOpType.add,
            )
            nc.vector._custom_dve(
                EXP2_FIXUP,
                out=pt[:, 1536 * grp + ACT_COLS: 1536 * grp + 1536],
                in0=uf16[:, STT_COLS * grp: STT_COLS * grp + STT_COLS],
                s0=MASK_CONST, s1=PC1, imm2=PC2,
            )

        pend.append((pt, va, wv, w))
        depth = 2 if w < nw - 2 else 1   # drain early so the tail is short
        while len(pend) > depth:
            emit_pv(nc, pend.pop(0))

    for st in pend:
        emit_pv(nc, st)
    ctx.close()


# --------------------------------------------------------------------------
# host side
# --------------------------------------------------------------------------
def _layer_norm(x, g, b, eps=1e-5):
    m = x.mean(-1, keepdims=True)
    v = x.var(-1, keepdims=True)
    return (x - m) / np.sqrt(v + eps) * g + b


def compute_bias(rpe_biases, rel_index, pos_proj_w, pos_proj_b, ln1_g, ln1_b,
                 fc1_w, fc1_b, ln2_g, ln2_b, fc2_w, fc2_b, ln3_g, ln3_b,
                 fc3_w, fc3_b):
    """pos-bias MLP + gather, in fp64 on host -> (6, 256, 256) [h, q, k]."""
    f8 = np.float64
    p = rpe_biases.astype(f8) @ pos_proj_w.astype(f8) + pos_proj_b.astype(f8)
    p = np.maximum(_layer_norm(p, ln1_g.astype(f8), ln1_b.astype(f8)), 0)
    p = p @ fc1_w.astype(f8) + fc1_b.astype(f8)
    p = np.maximum(_layer_norm(p, ln2_g.astype(f8), ln2_b.astype(f8)), 0)
    p = p @ fc2_w.astype(f8) + fc2_b.astype(f8)
    p = np.maximum(_layer_norm(p, ln3_g.astype(f8), ln3_b.astype(f8)), 0)
    pos = p @ fc3_w.astype(f8) + fc3_b.astype(f8)          # (num_biases, 6)
    rel = pos[np.asarray(rel_index).reshape(-1)]
    return np.ascontiguousarray(
        rel.reshape(N, N, NUM_HEADS).transpose(2, 0, 1)).astype(np.float64)


def im2win(x):
    """(B, L, C) -> (512, 256, C) windows in (b, hb, wb) / (hs, ws) order."""
    x = x.reshape(B, H // H_SP, H_SP, W // W_SP, W_SP, DIM)
    x = x.transpose(0, 1, 3, 2, 4, 5)
    return np.ascontiguousarray(x.reshape(NW_TOTAL, N, DIM))


def bias_banks(bias):
    """(6,256,256) fp64 [h,q,k] -> per-head [128, 512] LAM-scaled bank
    layout: col = 256*kc + q, partition = k_local."""
    bl = LAM * bias                                       # (6, 256, 256)
    bt = bl.transpose(0, 2, 1).reshape(NUM_HEADS, 2, 128, N)  # h, kc, k, q
    return np.ascontiguousarray(bt.transpose(0, 2, 1, 3)).reshape(
        NUM_HEADS, 128, 512)                               # fp64


def prep_inputs(qkv, bias):
    import ml_dtypes
    e4m3 = ml_dtypes.float8_e4m3

    q = im2win(np.asarray(qkv[0]))
    k = im2win(np.asarray(qkv[1]))
    v = im2win(np.asarray(qkv[2]))

    qTf = np.ascontiguousarray(
        (q * np.float32(SCALE * LAM)).transpose(2, 0, 1)).astype(np.float16)
    kTf = np.ascontiguousarray(k.transpose(2, 0, 1)).astype(np.float16)

    vr = v.reshape(NW_TOTAL, 2, 128, NUM_HEADS, HEAD_DIM)
    ones = np.ones((NW_TOTAL, 2, 128, NUM_HEADS, 1), np.float32)
    vAf = np.concatenate([vr, ones], -1)
    vAf = np.ascontiguousarray(
        vAf.reshape(NW_TOTAL, 2, 128, 198).transpose(2, 0, 1, 3)
    ).reshape(128, NW_TOTAL, 396).astype(np.float16)

    bb = bias_banks(bias)                                  # (6, 128, 512)
    # fp8 coarse+residual DoubleRow moving operands for the DR-covered
    # prefix of each group (banks 0,1 fully; bank 2 up to DR_COLS-1024),
    # and the fp16 in1 bias for the DVE tail (zeros where DR covered).
    drs, bls = [], []
    for grp in range(2):
        heads = (0, 1, 2) if grp == 0 else (3, 4, 5)
        drs.append(np.concatenate(
            [bb[heads[0]], bb[heads[1]],
             bb[heads[2]][:, :DR_COLS - 1024]], axis=1))
        tail = bb[heads[2]][:, ACT_COLS - 1024:].copy()    # [128, STT_COLS]
        tail[:, :DR_COLS - ACT_COLS] = 0.0
        bls.append(tail)
    biasDR = np.concatenate(drs, axis=1).astype(np.float16)
    biasL = np.concatenate(bls, axis=1).astype(np.float16)
    id2 = np.eye(128, dtype=np.float32).astype(np.float16)
    return qTf, kTf, vAf, biasDR, biasL, id2


def _run(qkv, rpe_biases, rel_index, params, trace=False, **spmd_kwargs):
    qkv = np.asarray(qkv, np.float32)
    bias = compute_bias(np.asarray(rpe_biases), np.asarray(rel_index), **params)
    qTf, kTf, vAf, biasDR, biasL, id2 = prep_inputs(qkv, bias)

    nc = build_program(NW)
    in_maps = []
    for c in range(N_CORES):
        s = slice(c * NW, (c + 1) * NW)
        in_maps.append({
            "qT": np.ascontiguousarray(qTf[:, s]).reshape(DIM, NW * N),
            "kT": np.ascontiguousarray(kTf[:, s]).reshape(DIM, NW * N),
            "vA": np.ascontiguousarray(vAf[:, s]).reshape(128, NW * 396),
            "biasDR": biasDR, "biasL": biasL, "id2": id2,
        })
    res = run_bass_kernel_spmd(nc, in_maps, core_ids=list(range(N_CORES)),
                               trace=trace, **spmd_kwargs)

    outw = np.stack([np.asarray(res.results[c]["outw"], np.float32)
                     for c in range(N_CORES)])
    x = outw.reshape(N_CORES, 128, NW, 2, DIM).transpose(0, 2, 3, 1, 4)
    return unwindow(x.reshape(NW_TOTAL, N, DIM)), res


def kernel(qkv, H=None, W=None, rpe_biases=None, rel_index=None, **params):
    return _run(qkv, rpe_biases, rel_index, params)[0]


def unwindow(x):
    """(512, 256, 192) -> (B, H, W, C)"""
    x = x.reshape(B, H // H_SP, W // W_SP, H_SP, W_SP, DIM)
    x = x.transpose(0, 1, 3, 2, 4, 5)
    return np.ascontiguousarray(x.reshape(B, H, W, DIM))
